# revision 71
# baseline (speedup 1.0000x reference)
"""AnchorTargetLayer (Faster R-CNN RPN) distributed Bass kernel for 8 TRN2 cores.

Strategy: shard the anchor axis T=H*W*9 across 8 cores (each core owns a
horizontal band of the image).  Per-core GT pruning: only the Mk GT boxes
that can geometrically overlap the band are kept (plus GT 0, padded with
far-away dummy boxes), cutting all O(T*M) work by M/Mk.

The per-pair ordering metric is g = inter / (a_area + g_area), computed in
raw f32 (g is strictly monotone in IoU, so max/argmax/column-max/equality
on g reproduce the reference's IoU comparisons; ties remain exact-value
ties).  Division uses the 1-instruction approximate reciprocal.

Engine split per chunk of CH anchor tiles:
  DVE : x-overlap (min,min,add), inter=relu*relu (custom), S=aarea+garea,
        R=recip_fast(S), g=inter*R, first-argmax extraction via a custom
        eq(g,vmax)*(C-Idx) op + reduce, one-hot for the PE gather.
  Pool: y-overlap (min,min,add), per-anchor vmax reduce, per-GT column max
        accumulation, and the post-collective is_best sweep.
  PE  : one-hot transpose + [Mk,4] GT-parameter gather matmuls (psum-grouped),
        and the tiny scatter/gather matmuls around the column-max AllReduce.

Collectives: AllReduce(max) of the per-GT column max ([128,1] f32 after an
on-core partition reduce + scatter to full-M), and one 8KB AllGather of
per-lane top-8 fg/bg sampling priorities.  The exact global 128th-largest
selection runs on the gathered top-8 candidates (the global top-130 of T iid
uniforms has <=8 members per lane w.h.p.), so the Q7 kth_largest scan is
O(8/lane) instead of O(1800/lane).
"""

import os
import numpy as np

import concourse.bass as bass
import concourse.bacc as bacc
import concourse.mybir as mybir
import concourse.bass_isa as bass_isa
import concourse.tile as tile
from concourse import masks
from concourse.bass_utils import run_bass_kernel_spmd

ALU = mybir.AluOpType
AF = mybir.ActivationFunctionType
F32 = mybir.dt.float32
AX = mybir.AxisListType

RPN_NEG_OV = 0.3
RPN_POS_OV = 0.7
NUM_FG = 128
LAD_LO = -1.0005
LAD_ST = 1.0015 / 64.0
M = 128          # number of GT boxes
A = 9            # anchors per position
BIG_AREA = 1.0e30
THR_FG = float(np.float32(0.7 / 1.7))   # g-space fg threshold
THR_BG = float(np.float32(0.3 / 1.3))   # g-space bg threshold

NPL = 15         # anchor-coefficient planes

# ---------------------------------------------------------------------------
# custom DVE ops (registered into concourse.dve_ops at import)
# ---------------------------------------------------------------------------


def _relu_mul_ref(in0, in1, c0, c1, c2):
    a = np.maximum(np.nan_to_num(np.asarray(in0, np.float32), nan=0.0), 0)
    b = np.maximum(np.nan_to_num(np.asarray(in1, np.float32), nan=0.0), 0)
    return (a * b).astype(np.float32)


def _eq_idx_ref(in0, in1, c0, c1, c2):
    x = np.asarray(in0, np.float32)
    P = x.shape[0]
    xf = x.reshape(P, -1)
    y = np.asarray(in1, np.float32).reshape(P, -1)
    if y.shape[1] != xf.shape[1]:
        assert xf.shape[1] % y.shape[1] == 0
        y = np.repeat(y, xf.shape[1] // y.shape[1], axis=1)
    yf = y
    n = xf.shape[1]
    idx = np.arange(n, dtype=np.float32)[None, :]
    c0v = np.asarray(c0, np.float32).reshape(-1, 1) if isinstance(c0, np.ndarray) else np.float32(c0)
    out = (xf == yf).astype(np.float32) * (c0v - idx)
    return out.reshape(x.shape).astype(np.float32)


def _register_custom_ops():
    from concourse import dve_ops as D
    from concourse.dve_spec import Spec, Src0, Src1, C0, relu, eq, lower, Idx
    from concourse.dve_uop import DveOpSpec

    def reg(name, spec):
        if name in D._SUB_OPCODE_FOR_NAME:
            return next(op for op in D.OPS if op.name == name)
        shas = {}
        for ver in ("v3", "v4"):
            u = lower(spec, ver=ver)
            shas[ver] = DveOpSpec(name=name, opcode=1, uops=u,
                                  rd1_en=True).sha(ver)
        op = D.DveOp(name, spec, subdim=False, uops_sha=shas)
        D.OPS.append(op)
        D._SUB_OPCODE_FOR_NAME[name] = D._CUSTOM_DVE_ROW_BASE + len(D.OPS) - 1
        D.CUSTOM_DVE_SPECS[name] = spec
        return op

    from concourse.dve_spec import Zero
    rm = reg("ANT_ATL_RELU_MUL",
             Spec(body=relu(Src0) * relu(Src1), reference=_relu_mul_ref))
    ei = reg("ANT_ATL_EQ_IDX",
             Spec(body=eq(Src0, Src1) * (C0 - Idx), reference=_eq_idx_ref))
    ep = reg("ANT_ATL_EQ_POS",
             Spec(body=eq(Src0, Src1) * (Src0 > Zero), reference=_eq_pos_ref))
    return rm, ei, ep


def _eq_pos_ref(in0, in1, c0, c1, c2):
    x = np.asarray(in0, np.float32)
    P = x.shape[0]
    xf = x.reshape(P, -1)
    y = np.asarray(in1, np.float32).reshape(P, -1)
    if y.shape[1] != xf.shape[1]:
        assert xf.shape[1] % y.shape[1] == 0
        y = np.repeat(y, xf.shape[1] // y.shape[1], axis=1)
    out = (xf == y).astype(np.float32) * (xf > 0).astype(np.float32)
    return out.reshape(x.shape).astype(np.float32)


_RELU_MUL, _EQ_IDX, _EQ_POS = _register_custom_ops()


def _bk(ap2d, CH):
    """[128, X] -> [128, CH, X] with a step-0 chunk dim (broadcast over k)."""
    return ap2d.rearrange("p (o j) -> p o j", o=1).broadcast_to(
        (128, CH, ap2d.shape[1]))


def _bj(ap2d, J):
    """[128, CH] -> [128, CH, J] with a step-0 inner dim (broadcast over j)."""
    return ap2d.rearrange("p (k o) -> p k o", o=1).broadcast_to(
        (128, ap2d.shape[1], J))


def _pick_ch(NT):
    for c in (15, 25, 9, 5, 45, 3, 1):
        if NT % c == 0 and c <= 25:
            return c
    return 1


def build_graph(H, W, n_cores, Mk=None):
    """Build the SPMD Bass graph for one core (all cores run the same graph)."""
    T = H * W * A
    TPC = T // n_cores
    NT = TPC // 128
    assert TPC % 128 == 0
    if Mk is None:
        Mk = 96 if H == 160 else M
    CH = _pick_ch(NT)
    NCH = NT // CH
    GSIZE = NT * Mk
    Q_SEL = 1.0 - (NUM_FG - 0.5) / (128 * 8 - 1)

    nc = bacc.Bacc(
        "TRN2", target_bir_lowering=False, debug=False,
        enable_asserts=False, num_devices=n_cores,
    )
    pool_eng = nc.vector if os.environ.get("KNOPOOL") else nc.gpsimd

    # ---- kernel I/O ----
    I16 = mybir.dt.int16
    acoef = nc.dram_tensor("acoef", [128, NPL * NT], F32,
                           kind="ExternalInput")
    gcoef = nc.dram_tensor("gcoef", [128, 5 * Mk], F32, kind="ExternalInput")
    gtabd = nc.dram_tensor("gtab", [Mk, 4], F32, kind="ExternalInput")
    invwd = nc.dram_tensor("invw", [128, M // 16], I16, kind="ExternalInput")
    keptwd = nc.dram_tensor("keptw", [128, Mk // 16], I16,
                            kind="ExternalInput")
    kbiad = nc.dram_tensor("kbias", [128, Mk], F32, kind="ExternalInput")
    gt0d = nc.dram_tensor("gt0", [1, 4], F32, kind="ExternalInput")
    lad1d = nc.dram_tensor("lad1", [128, 64], F32, kind="ExternalInput")
    iot64d = nc.dram_tensor("iot64", [128, 64], F32, kind="ExternalInput")
    outt = nc.dram_tensor("out", [128, NT * 7], F32, kind="ExternalOutput")
    dbg = None
    if os.environ.get("KDEBUG"):
        dbg = nc.dram_tensor("dbg", [4, 128, NT], F32, kind="ExternalOutput")
        dbg2 = nc.dram_tensor("dbg2", [2, 128, Mk], F32, kind="ExternalOutput")

    # ---- internal DRAM (collective bounce buffers) ----
    cm_in = nc.dram_tensor("cm_in", [1, M], F32)
    cm_out = nc.dram_tensor("cm_out", [1, M], F32, addr_space="Shared")
    ag_in = nc.dram_tensor("ag_in", [2, 128, 8], F32)
    ag_out = nc.dram_tensor("ag_out", [n_cores, 2, 128, 8], F32,
                            addr_space="Shared")

    rg = [list(range(n_cores))]

    import contextlib
    with tile.TileContext(nc) as tc:
        with (
            tc.tile_pool(name="const", bufs=1) as cpool,
            tc.tile_pool(name="gbig", bufs=1) as gpool,
            tc.tile_pool(name="cols", bufs=1) as colp,
        ):
            # phase-1/2 pools live in an ExitStack, freed before the tail
            _ph = contextlib.ExitStack()
            work = _ph.enter_context(tc.tile_pool(name="work", bufs=1))
            ywork = _ph.enter_context(tc.tile_pool(name="ywork", bufs=1))
            xwork = _ph.enter_context(tc.tile_pool(name="xwork", bufs=2))
            ohp = _ph.enter_context(tc.tile_pool(name="ohp", bufs=2))
            pstp = _ph.enter_context(tc.tile_pool(name="pst", bufs=2,
                                                  space="PSUM"))
            gpsp = _ph.enter_context(tc.tile_pool(name="gps", bufs=2,
                                                  space="PSUM"))
            # ---- load constants / coefficients (single bulk DMAs) ----
            coefall = cpool.tile([128, NPL * NT], F32, tag="coefall")
            nc.sync.dma_start(coefall[:], acoef[:])
            coef = [coefall[:, i * NT:(i + 1) * NT] for i in range(NPL)]
            (nax1c, ax2pc, nay1c, ay2pc, aareac, insidec, invewc, invehc,
             ecxc, ecyc, logewc, logehc, ckmc, nrfgc, nrbgc) = coef

            gall = cpool.tile([128, 5 * Mk], F32, tag="gall")
            nc.sync.dma_start(gall[:], gcoef[:])
            gtt = [gall[:, i * Mk:(i + 1) * Mk] for i in range(5)]
            ngx1t, gx2pt, ngy1t, gy2pt, gareat = gtt

            gtabt = cpool.tile([Mk, 4], F32, tag="gtab")
            nc.sync.dma_start(gtabt[:], gtabd[:])
            invwt = cpool.tile([128, M // 16], I16, tag="invw")
            nc.sync.dma_start(invwt[:], invwd[:])
            keptwt = cpool.tile([128, Mk // 16], I16, tag="keptw")
            nc.sync.dma_start(keptwt[:], keptwd[:])
            kbiat = cpool.tile([128, Mk], F32, tag="kbia")
            nc.sync.dma_start(kbiat[:], kbiad[:])
            gt0r = cpool.tile([1, 4], F32, tag="gt0r")
            nc.sync.dma_start(gt0r[:], gt0d[:])
            gt0b = cpool.tile([128, 4], F32, tag="gt0b")
            nc.gpsimd.partition_broadcast(gt0b[:], gt0r[:], channels=128)
            lad1t = cpool.tile([128, 64], F32, tag="lad1")
            nc.sync.dma_start(lad1t[:], lad1d[:])
            iot64t = cpool.tile([128, 64], F32, tag="iot64")
            nc.sync.dma_start(iot64t[:], iot64d[:])

            identb = cpool.tile([128, 128], F32, tag="identb")
            masks.make_identity(nc, identb[:])

            # GT-side broadcast views (same for every chunk)
            ngx1b = _bk(ngx1t[:], CH)
            gx2pb = _bk(gx2pt[:], CH)
            ngy1b = _bk(ngy1t[:], CH)
            gy2pb = _bk(gy2pt[:], CH)
            gareab = _bk(gareat[:], CH)

            gbuf_t = gpool.tile([128, GSIZE], F32, tag="g")
            vmaxb = colp.tile([128, NT], F32, tag="vmaxb")
            isbb = colp.tile([128, NT], F32, tag="isbb")
            cmk = colp.tile([128, Mk], F32, tag="cmk")
            gres = colp.tile([128, NT * 4], F32, tag="gres")

            # ---- phase 1: g matrix, row max, first-argmax, PE gather ----
            # Software-pipelined: chunk c's g consumers (vmax/colmax/one-hot)
            # are emitted after chunk c+1's producers, so DVE never stalls on
            # the Pool-engine g-multiply handoff.
            def produce_front(c):
                # mins on DVE; both overlap adds + S on Pool
                k0 = c * CH
                nax1b = _bj(nax1c[:, k0:k0 + CH], Mk)
                ax2pb = _bj(ax2pc[:, k0:k0 + CH], Mk)
                nay1b = _bj(nay1c[:, k0:k0 + CH], Mk)
                ay2pb = _bj(ay2pc[:, k0:k0 + CH], Mk)
                aareab = _bj(aareac[:, k0:k0 + CH], Mk)

                m3 = ywork.tile([128, CH, Mk], F32, tag="m3")
                nc.vector.tensor_tensor(m3[:], nay1b, ngy1b, op=ALU.min)
                m4 = ywork.tile([128, CH, Mk], F32, tag="m4")
                nc.vector.tensor_tensor(m4[:], ay2pb, gy2pb, op=ALU.min)
                ih = ywork.tile([128, CH, Mk], F32, tag="ih")
                pool_eng.tensor_tensor(ih[:], m3[:], m4[:], op=ALU.add)
                m1 = work.tile([128, CH, Mk], F32, tag="m1")
                nc.vector.tensor_tensor(m1[:], nax1b, ngx1b, op=ALU.min)
                m2 = work.tile([128, CH, Mk], F32, tag="m2")
                nc.vector.tensor_tensor(m2[:], ax2pb, gx2pb, op=ALU.min)
                iw = ywork.tile([128, CH, Mk], F32, tag="iw")
                pool_eng.tensor_tensor(iw[:], m1[:], m2[:], op=ALU.add)
                su = xwork.tile([128, CH, Mk], F32, tag="su")
                pool_eng.tensor_tensor(su[:], aareab, gareab, op=ALU.add)
                return iw, ih, su, k0

            def produce_back(state):
                iw, ih, su, k0 = state
                inter = ywork.tile([128, CH, Mk], F32, tag="it")
                nc.vector._custom_dve(_RELU_MUL, out=inter[:], in0=iw[:],
                                      in1=ih[:])
                rr = xwork.tile([128, CH, Mk], F32, tag="rr")
                nc.vector.reciprocal_approx_fast(out=rr[:], in_=su[:])
                gv = gbuf_t[:, k0 * Mk:(k0 + CH) * Mk].rearrange(
                    "p (k j) -> p k j", j=Mk)
                pool_eng.tensor_tensor(gv, inter[:], rr[:], op=ALU.mult)

            def consume(c):
                k0 = c * CH
                gv = gbuf_t[:, k0 * Mk:(k0 + CH) * Mk].rearrange(
                    "p (k j) -> p k j", j=Mk)
                nc.vector.reduce_max(vmaxb[:, k0:k0 + CH], gv, axis=AX.X)
                gvt = gbuf_t[:, k0 * Mk:(k0 + CH) * Mk].rearrange(
                    "p (k j) -> p j k", j=Mk)
                if c == 0:
                    nc.vector.reduce_max(cmk[:], gvt, axis=AX.X)
                else:
                    tcm = ywork.tile([128, Mk], F32, tag="tcm")
                    nc.vector.reduce_max(tcm[:], gvt, axis=AX.X)
                    nc.vector.tensor_tensor(cmk[:], cmk[:], tcm[:],
                                            op=ALU.max)

                # one-hot in a single pass: (g == vmax) & (g > 0); rows with
                # no positive overlap get no hot (GT0 targets blended later)
                ohc = ohp.tile([128, CH, Mk], F32, tag="OH")
                nc.vector._custom_dve(
                    _EQ_POS, out=ohc[:], in0=gv,
                    in1=_bj(vmaxb[:, k0:k0 + CH], Mk))
                gps = gpsp.tile([128, 4 * CH], F32, tag="gps")
                for t in range(CH):
                    pst = pstp.tile([Mk, 128], F32, tag="pst")
                    nc.tensor.transpose(pst[:], ohc[:, t, :], identb[:])
                    ohT = ohp.tile([Mk, 128], F32, tag="ohT")
                    nc.scalar.copy(ohT[:], pst[:])
                    nc.tensor.matmul(gps[:, 4 * t:4 * (t + 1)], ohT[:],
                                     gtabt[:], start=True, stop=True)
                nc.scalar.copy(gres[:, k0 * 4:(k0 + CH) * 4], gps[:])

            st = produce_front(0)
            produce_back(st)
            for c in range(1, NCH):
                st = produce_front(c)
                consume(c - 1)
                produce_back(st)
            consume(NCH - 1)

            # ---- global per-GT max: partition reduce, exact scatter to the
            # full M columns (ap_gather with an inverse index map + sentinel),
            # AllReduce(max), exact gather back to kept columns ----
            cmka = colp.tile([128, Mk], F32, tag="cmka")
            nc.gpsimd.partition_all_reduce(cmka[:], cmk[:], channels=128,
                                           reduce_op=bass_isa.ReduceOp.max)
            cmext = colp.tile([128, Mk + 16], F32, tag="cmext")
            nc.vector.tensor_copy(cmext[:, 0:Mk], cmka[:])
            nc.vector.tensor_scalar(cmext[:, Mk:Mk + 16], cmka[:, 0:16],
                                    0.0, -BIG_AREA, op0=ALU.mult, op1=ALU.add)
            cfull = colp.tile([128, M], F32, tag="cfull")
            nc.gpsimd.ap_gather(cfull[:], cmext[:], invwt[:], channels=128,
                                num_elems=Mk + 16, d=1, num_idxs=M)
            nc.sync.dma_start(cm_in[:], cfull[0:1, :])
            nc.gpsimd.collective_compute(
                "AllReduce", ALU.max, replica_groups=rg,
                ins=[cm_in[:].opt()], outs=[cm_out[:].opt()])
            g1 = colp.tile([1, M], F32, tag="g1")
            nc.sync.dma_start(g1[:], cm_out[:])
            gfb = colp.tile([128, M], F32, tag="gfb")
            nc.gpsimd.partition_broadcast(gfb[:], g1[:], channels=128)
            cmaxt = colp.tile([128, Mk], F32, tag="cmaxt")
            nc.gpsimd.ap_gather(cmaxt[:], gfb[:], keptwt[:], channels=128,
                                num_elems=M, d=1, num_idxs=Mk)
            nc.vector.tensor_tensor(cmaxt[:], cmaxt[:], kbiat[:], op=ALU.add)
            cmaxb = _bk(cmaxt[:], CH)

            # ---- phase 2: is_best sweep (Pool) ----
            for c in range(NCH):
                k0 = c * CH
                gv = gbuf_t[:, k0 * Mk:(k0 + CH) * Mk].rearrange(
                    "p (k j) -> p k j", j=Mk)
                ee = ywork.tile([128, CH, Mk], F32,
                                tag="m3" if c % 2 == 0 else "m4")
                nc.vector.tensor_tensor(ee[:], gv, cmaxb, op=ALU.is_equal)
                nc.vector.reduce_max(isbb[:, k0:k0 + CH], ee[:], axis=AX.X)

            if dbg is not None:
                nc.sync.dma_start(dbg[0], vmaxb[:])
                nc.sync.dma_start(dbg[1], isbb[:])
                nc.sync.dma_start(dbg[2], vmaxb[:])
                nc.sync.dma_start(dbg[3], isbb[:])
                nc.sync.dma_start(dbg2[0], cmaxt[:])
                nc.sync.dma_start(dbg2[1], cmka[:])

            _ph.close()  # free phase pools before the tail allocations

            # ---- labels + priorities ----
            fgm = colp.tile([128, NT], F32, tag="fgm")
            tvf = colp.tile([128, NT], F32, tag="tvf")
            nc.vector.tensor_scalar(tvf[:], vmaxb[:], THR_FG, None,
                                    op0=ALU.is_ge)
            nc.vector.tensor_tensor(fgm[:], tvf[:], isbb[:], op=ALU.max)
            bgm0 = colp.tile([128, NT], F32, tag="bgm0")
            nc.vector.scalar_tensor_tensor(bgm0[:], vmaxb[:], THR_BG,
                                           insidec[:], op0=ALU.is_lt,
                                           op1=ALU.mult)
            nfgm = colp.tile([128, NT], F32, tag="nfgm")
            nc.vector.tensor_scalar(nfgm[:], fgm[:], -1.0, 1.0,
                                    op0=ALU.mult, op1=ALU.add)
            bgm = colp.tile([128, NT], F32, tag="bgm")
            nc.vector.tensor_tensor(bgm[:], bgm0[:], nfgm[:], op=ALU.mult)

            prfg = colp.tile([128, NT], F32, tag="prfg")
            s1 = colp.tile([128, NT], F32, tag="s1")
            nc.vector.scalar_tensor_tensor(s1[:], nrfgc[:], 2.0, fgm[:],
                                           op0=ALU.add, op1=ALU.mult)
            nc.vector.tensor_scalar(prfg[:], s1[:], -2.0, None, op0=ALU.add)
            prbg = colp.tile([128, NT], F32, tag="prbg")
            s2 = colp.tile([128, NT], F32, tag="s2")
            nc.vector.scalar_tensor_tensor(s2[:], nrbgc[:], 2.0, bgm[:],
                                           op0=ALU.add, op1=ALU.mult)
            nc.vector.tensor_scalar(prbg[:], s2[:], -2.0, None, op0=ALU.add)

            # ---- top-8 per lane, AllGather candidates, kth thresholds ----
            fg8 = colp.tile([128, 8], F32, tag="fg8")
            nc.vector.max(fg8[:], prfg[:])
            bg8 = colp.tile([128, 8], F32, tag="bg8")
            nc.vector.max(bg8[:], prbg[:])
            nc.sync.dma_start(ag_in[0], fg8[:])
            nc.sync.dma_start(ag_in[1], bg8[:])
            nc.gpsimd.collective_compute(
                "AllGather", ALU.bypass, replica_groups=rg,
                ins=[ag_in[:].opt()], outs=[ag_out[:].opt()])

            # gathered candidates: fg in columns 0:64, bg in 64:128
            cboth = colp.tile([128, 16 * n_cores], F32, tag="cboth")
            for r in range(n_cores):
                nc.sync.dma_start(cboth[:, r * 8:(r + 1) * 8], ag_out[r, 0])
                nc.sync.dma_start(cboth[:, 64 + r * 8:64 + (r + 1) * 8],
                                  ag_out[r, 1])

            # exact 128th-largest threshold by two rounds of ladder counting
            # (counts are global: every core holds all 1024 candidates)
            LO = LAD_LO
            ST = LAD_ST
            ST2 = ST / 64.0

            def extract_tau(cnta, base_is_lad1, tau_lo, tag):
                # cnta [128, 64] all-reduced counts -> tau [128, 1]
                mm = colp.tile([128, 64], F32, tag=f"mm{tag}")
                nc.vector.tensor_scalar(mm[:], cnta, float(NUM_FG), None,
                                        op0=ALU.is_ge)
                nc.vector.tensor_tensor(mm[:], mm[:], iot64t[:], op=ALU.mult)
                ii = colp.tile([128, 1], F32, tag=f"ii{tag}")
                nc.vector.reduce_max(ii[:], mm[:], axis=AX.X)
                tau = colp.tile([128, 1], F32, tag=f"tau{tag}")
                if base_is_lad1:
                    nc.vector.tensor_scalar(tau[:], ii[:], ST, LO,
                                            op0=ALU.mult, op1=ALU.add)
                else:
                    nc.vector.tensor_scalar(tau[:], ii[:], ST2,
                                            tau_lo[:, 0:1],
                                            op0=ALU.mult, op1=ALU.add)
                return tau

            with tc.tile_pool(name="ladp", bufs=1) as ladp:
                c1 = colp.tile([128, 128], F32, tag="c1")
                cmp1 = ladp.tile([128, 64, 64], F32, tag="cmp")
                nc.vector.tensor_tensor(cmp1[:], _bk(cboth[:, 0:64], 64),
                                        _bj(lad1t[:], 64), op=ALU.is_ge)
                nc.vector.reduce_sum(c1[:, 0:64], cmp1[:], axis=AX.X)
                cmp1b = ladp.tile([128, 64, 64], F32, tag="cmp")
                nc.vector.tensor_tensor(cmp1b[:], _bk(cboth[:, 64:128], 64),
                                        _bj(lad1t[:], 64), op=ALU.is_ge)
                nc.vector.reduce_sum(c1[:, 64:128], cmp1b[:], axis=AX.X)
                c1a = colp.tile([128, 128], F32, tag="c1a")
                nc.gpsimd.partition_all_reduce(
                    c1a[:], c1[:], channels=128,
                    reduce_op=bass_isa.ReduceOp.add)
                tlo_f = extract_tau(c1a[:, 0:64], True, None, "f1")
                tlo_b = extract_tau(c1a[:, 64:128], True, None, "b1")

                lad2f = colp.tile([128, 64], F32, tag="lad2f")
                nc.vector.tensor_scalar(lad2f[:], iot64t[:], ST2,
                                        tlo_f[:, 0:1], op0=ALU.mult,
                                        op1=ALU.add)
                lad2b = colp.tile([128, 64], F32, tag="lad2b")
                nc.vector.tensor_scalar(lad2b[:], iot64t[:], ST2,
                                        tlo_b[:, 0:1], op0=ALU.mult,
                                        op1=ALU.add)
                c2 = colp.tile([128, 128], F32, tag="c2")
                cmp2 = ladp.tile([128, 64, 64], F32, tag="cmp")
                nc.vector.tensor_tensor(cmp2[:], _bk(cboth[:, 0:64], 64),
                                        _bj(lad2f[:], 64), op=ALU.is_ge)
                nc.vector.reduce_sum(c2[:, 0:64], cmp2[:], axis=AX.X)
                cmp2b = ladp.tile([128, 64, 64], F32, tag="cmp")
                nc.vector.tensor_tensor(cmp2b[:], _bk(cboth[:, 64:128], 64),
                                        _bj(lad2b[:], 64), op=ALU.is_ge)
                nc.vector.reduce_sum(c2[:, 64:128], cmp2b[:], axis=AX.X)
                c2a = colp.tile([128, 128], F32, tag="c2a")
                nc.gpsimd.partition_all_reduce(
                    c2a[:], c2[:], channels=128,
                    reduce_op=bass_isa.ReduceOp.add)
                thfgb = extract_tau(c2a[:, 0:64], False, tlo_f, "f2")
                thbgb = extract_tau(c2a[:, 64:128], False, tlo_b, "b2")

            # counts over the gathered candidate sets -> 1 / num_examples
            mcf = colp.tile([128, 64], F32, tag="mcf")
            nc.vector.tensor_scalar(mcf[:], cboth[:, 0:64], thfgb[:, 0:1],
                                    None, op0=ALU.is_ge)
            nf1 = colp.tile([128, 1], F32, tag="nf1")
            nc.vector.reduce_sum(nf1[:], mcf[:], axis=AX.X)
            nfk = colp.tile([128, 1], F32, tag="nfk")
            nc.gpsimd.partition_all_reduce(nfk[:], nf1[:], channels=128,
                                           reduce_op=bass_isa.ReduceOp.add)
            mcb = colp.tile([128, 64], F32, tag="mcb")
            nc.vector.tensor_scalar(mcb[:], cboth[:, 64:128], thbgb[:, 0:1],
                                    None, op0=ALU.is_ge)
            nb1 = colp.tile([128, 1], F32, tag="nb1")
            nc.vector.reduce_sum(nb1[:], mcb[:], axis=AX.X)
            nbk = colp.tile([128, 1], F32, tag="nbk")
            nc.gpsimd.partition_all_reduce(nbk[:], nb1[:], channels=128,
                                           reduce_op=bass_isa.ReduceOp.add)
            numex = colp.tile([128, 1], F32, tag="numex")
            nc.vector.tensor_tensor(numex[:], nfk[:], nbk[:], op=ALU.add)
            invne = colp.tile([128, 1], F32, tag="invne")
            nc.vector.reciprocal(invne[:], numex[:])

            # ---- phase 3: final labels / weights / bbox targets ----
            mfg = colp.tile([128, NT], F32, tag="mfg")
            nc.vector.tensor_scalar(mfg[:], prfg[:], thfgb[:, 0:1], None,
                                    op0=ALU.is_ge)
            mbg = colp.tile([128, NT], F32, tag="mbg")
            nc.vector.tensor_scalar(mbg[:], prbg[:], thbgb[:, 0:1], None,
                                    op0=ALU.is_ge)
            labf = colp.tile([128, NT], F32, tag="labf")
            nc.vector.scalar_tensor_tensor(labf[:], mfg[:], 2.0, mbg[:],
                                           op0=ALU.mult, op1=ALU.add)
            nc.vector.tensor_scalar(labf[:], labf[:], 1.0, None,
                                    op0=ALU.subtract)
            oww = colp.tile([128, NT], F32, tag="oww")
            nc.vector.tensor_tensor(oww[:], mfg[:], mbg[:], op=ALU.add)
            nc.vector.tensor_scalar(oww[:], oww[:], invne[:, 0:1], None,
                                    op0=ALU.mult)

            _tp = contextlib.ExitStack()
            resp = _tp.enter_context(tc.tile_pool(name="resp", bufs=1))
            res = resp.tile([128, NT * 7], F32, tag="res")
            r3 = res[:].rearrange("p (k c) -> p k c", c=7)
            g4 = gres[:].rearrange("p (k c) -> p k c", c=4)
            # zero-overlap rows have an all-zero one-hot; blend in GT0 params
            zs = colp.tile([128, NT], F32, tag="zs")
            nc.vector.tensor_scalar(zs[:], vmaxb[:], 0.0, None, op0=ALU.is_gt)
            nzs = colp.tile([128, NT], F32, tag="nzs")
            nc.vector.tensor_scalar(nzs[:], zs[:], -1.0, 1.0,
                                    op0=ALU.mult, op1=ALU.add)
            tb1 = colp.tile([128, NT], F32, tag="tb1")
            for cc in range(4):
                nc.vector.tensor_scalar(tb1[:], nzs[:], gt0b[:, cc:cc + 1],
                                        None, op0=ALU.mult)
                nc.vector.tensor_tensor(g4[:, :, cc], g4[:, :, cc], zs[:],
                                        op=ALU.mult)
                nc.vector.tensor_tensor(g4[:, :, cc], g4[:, :, cc], tb1[:],
                                        op=ALU.add)
            tmp = colp.tile([128, NT], F32, tag="tmp")
            nc.vector.tensor_tensor(tmp[:], g4[:, :, 0], ecxc[:],
                                    op=ALU.subtract)
            nc.vector.tensor_tensor(r3[:, :, 1], tmp[:], invewc[:],
                                    op=ALU.mult)
            nc.vector.tensor_tensor(tmp[:], g4[:, :, 1], ecyc[:],
                                    op=ALU.subtract)
            nc.vector.tensor_tensor(r3[:, :, 2], tmp[:], invehc[:],
                                    op=ALU.mult)
            nc.vector.tensor_tensor(r3[:, :, 3], g4[:, :, 2], logewc[:],
                                    op=ALU.subtract)
            nc.vector.tensor_tensor(r3[:, :, 4], g4[:, :, 3], logehc[:],
                                    op=ALU.subtract)
            for cc in range(4):
                nc.vector.tensor_tensor(r3[:, :, 1 + cc], r3[:, :, 1 + cc],
                                        insidec[:], op=ALU.mult)
            nc.vector.tensor_copy(r3[:, :, 0], labf[:])
            nc.vector.tensor_copy(r3[:, :, 5], mfg[:])
            nc.vector.tensor_copy(r3[:, :, 6], oww[:])

            nc.sync.dma_start(outt[:], res[:])
            _tp.close()

    nc.compile()
    return nc


def _kept_sets(all_anchors, gt, n_cores):
    T = all_anchors.shape[0]
    TPC = T // n_cores
    gx1, gy1, gx2, gy2 = gt[:, 0], gt[:, 1], gt[:, 2], gt[:, 3]
    sets = []
    for c in range(n_cores):
        sl = slice(c * TPC, (c + 1) * TPC)
        aa = all_anchors[sl]
        keep = ((gy2 + 1 > aa[:, 1].min()) & (gy1 < aa[:, 3].max() + 1)
                & (gx2 + 1 > aa[:, 0].min()) & (gx1 < aa[:, 2].max() + 1))
        keep[0] = True
        sets.append(np.nonzero(keep)[0])
    return sets


def plan_mk(rpn_cls_score, gt_boxes, anchors, feat_stride, n_cores):
    f32 = np.float32
    H, W = rpn_cls_score.shape[-2:]
    anchors = np.asarray(anchors, dtype=f32)
    fs = f32(feat_stride)
    sx = np.arange(W, dtype=f32) * fs
    sy = np.arange(H, dtype=f32) * fs
    gy, gx = np.meshgrid(sy, sx, indexing="ij")
    shifts = np.stack([gx.ravel(), gy.ravel(), gx.ravel(), gy.ravel()],
                      axis=1).astype(f32)
    all_anchors = (anchors[None, :, :] + shifts[:, None, :]).reshape(-1, 4)
    gt = np.asarray(gt_boxes, dtype=f32)
    sets = _kept_sets(all_anchors, gt, n_cores)
    mx = max(len(s) for s in sets)
    Mk = min(M, int(np.ceil(max(mx, 32) / 16.0) * 16))
    return Mk, all_anchors, sets


def prep_inputs(rpn_cls_score, gt_boxes, im_info, anchors, rand_fg, rand_bg,
                feat_stride, n_cores, Mk=None, all_anchors=None, ksets=None):
    """Host-side input marshalling."""
    f32 = np.float32
    H, W = rpn_cls_score.shape[-2:]
    T = H * W * A
    TPC = T // n_cores
    NT = TPC // 128
    CH = _pick_ch(NT)
    if Mk is None or all_anchors is None or ksets is None:
        Mk, all_anchors, ksets = plan_mk(rpn_cls_score, gt_boxes, anchors,
                                         feat_stride, n_cores)

    ax1, ay1, ax2, ay2 = (all_anchors[:, i] for i in range(4))
    im = np.asarray(im_info, dtype=f32)[0]
    inside = ((ax1 >= 0) & (ay1 >= 0) & (ax2 < im[1]) & (ay2 < im[0]))

    ew = ax2 - ax1 + f32(1.0)
    eh = ay2 - ay1 + f32(1.0)
    a_area = ew * eh
    a_area_eff = np.where(inside, a_area, f32(BIG_AREA)).astype(f32)
    ecx = ax1 + f32(0.5) * ew
    ecy = ay1 + f32(0.5) * eh
    ckm = np.tile((f32(CH * Mk)
                   - (np.arange(NT) % CH).astype(f32) * f32(Mk)), (128, 1))

    gt = np.asarray(gt_boxes, dtype=f32)
    gx1, gy1, gx2, gy2 = gt[:, 0], gt[:, 1], gt[:, 2], gt[:, 3]
    gw = gx2 - gx1 + f32(1.0)
    gh = gy2 - gy1 + f32(1.0)
    g_area = gw * gh
    gcx = gx1 + f32(0.5) * gw
    gcy = gy1 + f32(0.5) * gh
    loggw = np.log(gw).astype(f32)
    loggh = np.log(gh).astype(f32)

    rand_fg = np.asarray(rand_fg, dtype=f32)
    rand_bg = np.asarray(rand_bg, dtype=f32)

    in_maps = []
    for c in range(n_cores):
        sl = slice(c * TPC, (c + 1) * TPC)
        idx = ksets[c]
        nk = len(idx)
        assert nk <= Mk, f"core {c}: kept {nk} > Mk {Mk}"

        coefs = np.stack([
            -ax1[sl], ax2[sl] + f32(1.0), -ay1[sl], ay2[sl] + f32(1.0),
            a_area_eff[sl],
            inside[sl].astype(f32), (f32(1.0) / ew[sl]), (f32(1.0) / eh[sl]),
            ecx[sl], ecy[sl], np.log(ew[sl]), np.log(eh[sl]),
            np.zeros(TPC, f32),  # placeholder, replaced below
            (-rand_fg[sl]), (-rand_bg[sl]),
        ], axis=0).astype(f32).reshape(NPL, 128, NT)
        coefs[12] = ckm
        coefs = coefs.transpose(1, 0, 2).reshape(128, NPL * NT)

        kx1 = np.full(Mk, f32(-1e6)); kx2 = np.full(Mk, f32(-1e6 + 1))
        ky1 = np.full(Mk, f32(-1e6)); ky2 = np.full(Mk, f32(-1e6 + 1))
        kga = np.full(Mk, f32(BIG_AREA))
        kx1[:nk], kx2[:nk] = gx1[idx], gx2[idx]
        ky1[:nk], ky2[:nk] = gy1[idx], gy2[idx]
        kga[:nk] = g_area[idx]
        gcoefs = np.concatenate([
            np.tile(-kx1, (128, 1)), np.tile(kx2 + f32(1.0), (128, 1)),
            np.tile(-ky1, (128, 1)), np.tile(ky2 + f32(1.0), (128, 1)),
            np.tile(kga, (128, 1)),
        ], axis=1).astype(f32)

        gtab = np.zeros((Mk, 4), f32)
        gtab[:nk, 0] = gcx[idx]
        gtab[:nk, 1] = gcy[idx]
        gtab[:nk, 2] = loggw[idx]
        gtab[:nk, 3] = loggh[idx]

        # inverse map: full column j -> kept slot (or the -1e30 sentinel)
        inv_full = np.full(M, Mk, np.int16)
        inv_full[idx] = np.arange(nk, dtype=np.int16)
        kept_idx = np.zeros(Mk, np.int16)
        kept_idx[:nk] = idx.astype(np.int16)
        kbias = np.zeros((128, Mk), f32)
        kbias[:, nk:] = f32(-BIG_AREA)

        def wrap16(a):
            # ap_gather idx layout: position i -> idxs[i % 16, i // 16],
            # replicated across the 8 Q7 16-partition groups
            w = a.reshape(-1, 16).T.astype(np.int16)      # [16, n/16]
            return np.tile(w, (8, 1))

        in_maps.append({
            "acoef": np.ascontiguousarray(coefs),
            "gcoef": np.ascontiguousarray(gcoefs),
            "gtab": gtab,
            "invw": wrap16(inv_full),
            "keptw": wrap16(kept_idx),
            "kbias": kbias,
            "gt0": np.array([[gcx[0], gcy[0], loggw[0], loggh[0]]], f32),
            "lad1": np.tile((f32(LAD_LO)
                             + np.arange(64, dtype=f32) * f32(LAD_ST)),
                            (128, 1)),
            "iot64": np.tile(np.arange(64, dtype=f32), (128, 1)),
        })
    return in_maps


_GRAPH_CACHE = {}


def run(inputs, n_cores=8, trace=False):
    H, W = inputs["rpn_cls_score"].shape[-2:]
    Mk, all_anchors, ksets = plan_mk(inputs["rpn_cls_score"],
                                     inputs["gt_boxes"], inputs["anchors"],
                                     inputs["feat_stride"], n_cores)
    key = (H, W, n_cores, Mk)
    if key not in _GRAPH_CACHE:
        _GRAPH_CACHE[key] = build_graph(H, W, n_cores, Mk)
    nc = _GRAPH_CACHE[key]
    in_maps = prep_inputs(
        inputs["rpn_cls_score"], inputs["gt_boxes"], inputs["im_info"],
        inputs["anchors"], inputs["rand_fg"], inputs["rand_bg"],
        inputs["feat_stride"], n_cores, Mk, all_anchors, ksets)
    res = run_bass_kernel_spmd(nc, in_maps, core_ids=list(range(n_cores)),
                               trace=trace)
    T = H * W * A
    TPC = T // n_cores
    out = np.concatenate(
        [r["out"].reshape(TPC, 7) for r in res.results], axis=0)
    return out, res


def kernel(**inputs) -> np.ndarray:
    out, _ = run(inputs, n_cores=8, trace=False)
    return out


# revision 73
# speedup vs baseline: 1.3760x; 1.3760x over previous
"""AnchorTargetLayer (Faster R-CNN RPN) distributed Bass kernel for 8 TRN2 cores.

Strategy: shard the anchor axis T=H*W*9 across 8 cores (each core owns a
horizontal band of the image).  Per-core GT pruning: only the Mk GT boxes
that can geometrically overlap the band are kept (plus GT 0, padded with
far-away dummy boxes), cutting all O(T*M) work by M/Mk.

The per-pair ordering metric is g = inter / (a_area + g_area), computed in
raw f32 (g is strictly monotone in IoU, so max/argmax/column-max/equality
on g reproduce the reference's IoU comparisons; ties remain exact-value
ties).  Division uses the 1-instruction approximate reciprocal.

Engine split per chunk of CH anchor tiles:
  DVE : x-overlap (min,min,add), inter=relu*relu (custom), S=aarea+garea,
        R=recip_fast(S), g=inter*R, first-argmax extraction via a custom
        eq(g,vmax)*(C-Idx) op + reduce, one-hot for the PE gather.
  Pool: y-overlap (min,min,add), per-anchor vmax reduce, per-GT column max
        accumulation, and the post-collective is_best sweep.
  PE  : one-hot transpose + [Mk,4] GT-parameter gather matmuls (psum-grouped),
        and the tiny scatter/gather matmuls around the column-max AllReduce.

Collectives: AllReduce(max) of the per-GT column max ([128,1] f32 after an
on-core partition reduce + scatter to full-M), and one 8KB AllGather of
per-lane top-8 fg/bg sampling priorities.  The exact global 128th-largest
selection runs on the gathered top-8 candidates (the global top-130 of T iid
uniforms has <=8 members per lane w.h.p.), so the Q7 kth_largest scan is
O(8/lane) instead of O(1800/lane).
"""

import os
import numpy as np

import concourse.bass as bass
import concourse.bacc as bacc
import concourse.mybir as mybir
import concourse.bass_isa as bass_isa
import concourse.tile as tile
from concourse import masks
from concourse.bass_utils import run_bass_kernel_spmd

ALU = mybir.AluOpType
AF = mybir.ActivationFunctionType
F32 = mybir.dt.float32
AX = mybir.AxisListType

RPN_NEG_OV = 0.3
RPN_POS_OV = 0.7
NUM_FG = 128
LAD_LO = -1.0005
LAD_ST = 1.0015 / 64.0
M = 128          # number of GT boxes
A = 9            # anchors per position
BIG_AREA = 1.0e30
THR_FG = float(np.float32(0.7 / 1.7))   # g-space fg threshold
THR_BG = float(np.float32(0.3 / 1.3))   # g-space bg threshold

NPL = 15         # anchor-coefficient planes

# ---------------------------------------------------------------------------
# custom DVE ops (registered into concourse.dve_ops at import)
# ---------------------------------------------------------------------------


def _relu_mul_ref(in0, in1, c0, c1, c2):
    a = np.maximum(np.nan_to_num(np.asarray(in0, np.float32), nan=0.0), 0)
    b = np.maximum(np.nan_to_num(np.asarray(in1, np.float32), nan=0.0), 0)
    return (a * b).astype(np.float32)


def _eq_idx_ref(in0, in1, c0, c1, c2):
    x = np.asarray(in0, np.float32)
    P = x.shape[0]
    xf = x.reshape(P, -1)
    y = np.asarray(in1, np.float32).reshape(P, -1)
    if y.shape[1] != xf.shape[1]:
        assert xf.shape[1] % y.shape[1] == 0
        y = np.repeat(y, xf.shape[1] // y.shape[1], axis=1)
    yf = y
    n = xf.shape[1]
    idx = np.arange(n, dtype=np.float32)[None, :]
    c0v = np.asarray(c0, np.float32).reshape(-1, 1) if isinstance(c0, np.ndarray) else np.float32(c0)
    out = (xf == yf).astype(np.float32) * (c0v - idx)
    return out.reshape(x.shape).astype(np.float32)


def _register_custom_ops():
    from concourse import dve_ops as D
    from concourse.dve_spec import Spec, Src0, Src1, C0, relu, eq, lower, Idx
    from concourse.dve_uop import DveOpSpec

    def reg(name, spec):
        if name in D._SUB_OPCODE_FOR_NAME:
            return next(op for op in D.OPS if op.name == name)
        shas = {}
        for ver in ("v3", "v4"):
            u = lower(spec, ver=ver)
            shas[ver] = DveOpSpec(name=name, opcode=1, uops=u,
                                  rd1_en=True).sha(ver)
        op = D.DveOp(name, spec, subdim=False, uops_sha=shas)
        D.OPS.append(op)
        D._SUB_OPCODE_FOR_NAME[name] = D._CUSTOM_DVE_ROW_BASE + len(D.OPS) - 1
        D.CUSTOM_DVE_SPECS[name] = spec
        return op

    from concourse.dve_spec import Zero
    rm = reg("ANT_ATL_RELU_MUL",
             Spec(body=relu(Src0) * relu(Src1), reference=_relu_mul_ref))
    ei = reg("ANT_ATL_EQ_IDX",
             Spec(body=eq(Src0, Src1) * (C0 - Idx), reference=_eq_idx_ref))
    ep = reg("ANT_ATL_EQ_POS",
             Spec(body=eq(Src0, Src1) * (Src0 > Zero), reference=_eq_pos_ref))
    return rm, ei, ep


def _eq_pos_ref(in0, in1, c0, c1, c2):
    x = np.asarray(in0, np.float32)
    P = x.shape[0]
    xf = x.reshape(P, -1)
    y = np.asarray(in1, np.float32).reshape(P, -1)
    if y.shape[1] != xf.shape[1]:
        assert xf.shape[1] % y.shape[1] == 0
        y = np.repeat(y, xf.shape[1] // y.shape[1], axis=1)
    out = (xf == y).astype(np.float32) * (xf > 0).astype(np.float32)
    return out.reshape(x.shape).astype(np.float32)


_RELU_MUL, _EQ_IDX, _EQ_POS = _register_custom_ops()


def _bk(ap2d, CH):
    """[128, X] -> [128, CH, X] with a step-0 chunk dim (broadcast over k)."""
    return ap2d.rearrange("p (o j) -> p o j", o=1).broadcast_to(
        (128, CH, ap2d.shape[1]))


def _bj(ap2d, J):
    """[128, CH] -> [128, CH, J] with a step-0 inner dim (broadcast over j)."""
    return ap2d.rearrange("p (k o) -> p k o", o=1).broadcast_to(
        (128, ap2d.shape[1], J))


def _pick_ch(NT):
    for c in (15, 25, 9, 5, 45, 3, 1):
        if NT % c == 0 and c <= 25:
            return c
    return 1


def build_graph(H, W, n_cores, Mk=None):
    """Build the SPMD Bass graph for one core (all cores run the same graph)."""
    T = H * W * A
    TPC = T // n_cores
    NT = TPC // 128
    assert TPC % 128 == 0
    if Mk is None:
        Mk = 96 if H == 160 else M
    CH = _pick_ch(NT)
    NCH = NT // CH
    GSIZE = NT * Mk
    Q_SEL = 1.0 - (NUM_FG - 0.5) / (128 * 8 - 1)

    nc = bacc.Bacc(
        "TRN2", target_bir_lowering=False, debug=False,
        enable_asserts=False, num_devices=n_cores,
    )
    pool_eng = nc.vector if os.environ.get("KNOPOOL") else nc.gpsimd

    # ---- kernel I/O ----
    I16 = mybir.dt.int16
    acoef = nc.dram_tensor("acoef", [128, NPL * NT], F32,
                           kind="ExternalInput")
    gcoef = nc.dram_tensor("gcoef", [128, 5 * Mk], F32, kind="ExternalInput")
    gtabd = nc.dram_tensor("gtab", [Mk, 4], F32, kind="ExternalInput")
    invwd = nc.dram_tensor("invw", [128, M // 16], I16, kind="ExternalInput")
    keptwd = nc.dram_tensor("keptw", [128, Mk // 16], I16,
                            kind="ExternalInput")
    kbiad = nc.dram_tensor("kbias", [128, Mk], F32, kind="ExternalInput")
    gt0d = nc.dram_tensor("gt0", [1, 4], F32, kind="ExternalInput")
    lad1d = nc.dram_tensor("lad1", [128, 64], F32, kind="ExternalInput")
    iot64d = nc.dram_tensor("iot64", [128, 64], F32, kind="ExternalInput")
    outt = nc.dram_tensor("out", [128, NT * 7], F32, kind="ExternalOutput")
    dbg = None
    if os.environ.get("KDEBUG"):
        dbg = nc.dram_tensor("dbg", [4, 128, NT], F32, kind="ExternalOutput")
        dbg2 = nc.dram_tensor("dbg2", [2, 128, Mk], F32, kind="ExternalOutput")

    # ---- internal DRAM (collective bounce buffers) ----
    cm_in = nc.dram_tensor("cm_in", [1, M], F32)
    cm_out = nc.dram_tensor("cm_out", [1, M], F32, addr_space="Shared")
    ag_in = nc.dram_tensor("ag_in", [2, 128, 8], F32)
    ag_out = nc.dram_tensor("ag_out", [n_cores, 2, 128, 8], F32,
                            addr_space="Shared")

    rg = [list(range(n_cores))]

    import contextlib
    with tile.TileContext(nc) as tc:
        with (
            tc.tile_pool(name="const", bufs=1) as cpool,
            tc.tile_pool(name="gbig", bufs=1) as gpool,
            tc.tile_pool(name="cols", bufs=1) as colp,
        ):
            # phase-1/2 pools live in an ExitStack, freed before the tail
            _ph = contextlib.ExitStack()
            work = _ph.enter_context(tc.tile_pool(name="work", bufs=1))
            ywork = _ph.enter_context(tc.tile_pool(name="ywork", bufs=1))
            xwork = _ph.enter_context(tc.tile_pool(name="xwork", bufs=2))
            ohp = _ph.enter_context(tc.tile_pool(name="ohp", bufs=2))
            pstp = _ph.enter_context(tc.tile_pool(name="pst", bufs=2,
                                                  space="PSUM"))
            gpsp = _ph.enter_context(tc.tile_pool(name="gps", bufs=2,
                                                  space="PSUM"))
            # ---- load constants / coefficients (single bulk DMAs) ----
            coefall = cpool.tile([128, NPL * NT], F32, tag="coefall")
            nc.sync.dma_start(coefall[:], acoef[:])
            coef = [coefall[:, i * NT:(i + 1) * NT] for i in range(NPL)]
            (nax1c, ax2pc, nay1c, ay2pc, aareac, insidec, invewc, invehc,
             ecxc, ecyc, logewc, logehc, ckmc, nrfgc, nrbgc) = coef

            gall = cpool.tile([128, 5 * Mk], F32, tag="gall")
            nc.sync.dma_start(gall[:], gcoef[:])
            gtt = [gall[:, i * Mk:(i + 1) * Mk] for i in range(5)]
            ngx1t, gx2pt, ngy1t, gy2pt, gareat = gtt

            gtabt = cpool.tile([Mk, 4], F32, tag="gtab")
            nc.sync.dma_start(gtabt[:], gtabd[:])
            invwt = cpool.tile([128, M // 16], I16, tag="invw")
            nc.sync.dma_start(invwt[:], invwd[:])
            keptwt = cpool.tile([128, Mk // 16], I16, tag="keptw")
            nc.sync.dma_start(keptwt[:], keptwd[:])
            kbiat = cpool.tile([128, Mk], F32, tag="kbia")
            nc.sync.dma_start(kbiat[:], kbiad[:])
            gt0r = cpool.tile([1, 4], F32, tag="gt0r")
            nc.sync.dma_start(gt0r[:], gt0d[:])
            gt0b = cpool.tile([128, 4], F32, tag="gt0b")
            nc.gpsimd.partition_broadcast(gt0b[:], gt0r[:], channels=128)
            lad1t = cpool.tile([128, 64], F32, tag="lad1")
            nc.sync.dma_start(lad1t[:], lad1d[:])
            iot64t = cpool.tile([128, 64], F32, tag="iot64")
            nc.sync.dma_start(iot64t[:], iot64d[:])

            identb = cpool.tile([128, 128], F32, tag="identb")
            masks.make_identity(nc, identb[:])

            # GT-side broadcast views (same for every chunk)
            ngx1b = _bk(ngx1t[:], CH)
            gx2pb = _bk(gx2pt[:], CH)
            ngy1b = _bk(ngy1t[:], CH)
            gy2pb = _bk(gy2pt[:], CH)
            gareab = _bk(gareat[:], CH)

            gbuf_t = gpool.tile([128, GSIZE], F32, tag="g")
            vmaxb = colp.tile([128, NT], F32, tag="vmaxb")
            isbb = colp.tile([128, NT], F32, tag="isbb")
            cmk = colp.tile([128, Mk], F32, tag="cmk")
            gres = colp.tile([128, NT * 4], F32, tag="gres")

            # ---- phase 1: g matrix, row max, first-argmax, PE gather ----
            # Software-pipelined: chunk c's g consumers (vmax/colmax/one-hot)
            # are emitted after chunk c+1's producers, so DVE never stalls on
            # the Pool-engine g-multiply handoff.
            def produce(c):
                k0 = c * CH
                nax1b = _bj(nax1c[:, k0:k0 + CH], Mk)
                ax2pb = _bj(ax2pc[:, k0:k0 + CH], Mk)
                nay1b = _bj(nay1c[:, k0:k0 + CH], Mk)
                ay2pb = _bj(ay2pc[:, k0:k0 + CH], Mk)
                aareab = _bj(aareac[:, k0:k0 + CH], Mk)

                # y-overlap first; the add runs on Pool while DVE does x
                m3 = ywork.tile([128, CH, Mk], F32, tag="m3")
                nc.vector.tensor_tensor(m3[:], nay1b, ngy1b, op=ALU.min)
                m4 = ywork.tile([128, CH, Mk], F32, tag="m4")
                nc.vector.tensor_tensor(m4[:], ay2pb, gy2pb, op=ALU.min)
                ih = ywork.tile([128, CH, Mk], F32, tag="ih")
                pool_eng.tensor_tensor(ih[:], m3[:], m4[:], op=ALU.add)
                # x-overlap on DVE (iw accumulates in-place into m1)
                m1 = work.tile([128, CH, Mk], F32, tag="m1")
                nc.vector.tensor_tensor(m1[:], nax1b, ngx1b, op=ALU.min)
                m2 = work.tile([128, CH, Mk], F32, tag="m2")
                nc.vector.tensor_tensor(m2[:], ax2pb, gx2pb, op=ALU.min)
                nc.vector.tensor_tensor(m1[:], m1[:], m2[:], op=ALU.add)

                inter = ywork.tile([128, CH, Mk], F32, tag="it")
                nc.vector._custom_dve(_RELU_MUL, out=inter[:], in0=m1[:],
                                      in1=ih[:])
                su = xwork.tile([128, CH, Mk], F32, tag="su")
                pool_eng.tensor_tensor(su[:], aareab, gareab, op=ALU.add)
                rr = xwork.tile([128, CH, Mk], F32, tag="rr")
                nc.vector.reciprocal_approx_fast(out=rr[:], in_=su[:])
                gv = gbuf_t[:, k0 * Mk:(k0 + CH) * Mk].rearrange(
                    "p (k j) -> p k j", j=Mk)
                pool_eng.tensor_tensor(gv, inter[:], rr[:], op=ALU.mult)

            def consume(c):
                k0 = c * CH
                gv = gbuf_t[:, k0 * Mk:(k0 + CH) * Mk].rearrange(
                    "p (k j) -> p k j", j=Mk)
                nc.vector.reduce_max(vmaxb[:, k0:k0 + CH], gv, axis=AX.X)
                gvt = gbuf_t[:, k0 * Mk:(k0 + CH) * Mk].rearrange(
                    "p (k j) -> p j k", j=Mk)
                if c == 0:
                    nc.vector.reduce_max(cmk[:], gvt, axis=AX.X)
                else:
                    tcm = ywork.tile([128, Mk], F32, tag="tcm")
                    nc.vector.reduce_max(tcm[:], gvt, axis=AX.X)
                    nc.vector.tensor_tensor(cmk[:], cmk[:], tcm[:],
                                            op=ALU.max)

                # one-hot in a single pass: (g == vmax) & (g > 0); rows with
                # no positive overlap get no hot (GT0 targets blended later)
                ohc = ohp.tile([128, CH, Mk], F32, tag="OH")
                nc.vector._custom_dve(
                    _EQ_POS, out=ohc[:], in0=gv,
                    in1=_bj(vmaxb[:, k0:k0 + CH], Mk))
                gps = gpsp.tile([128, 4 * CH], F32, tag="gps")
                for t in range(CH):
                    pst = pstp.tile([Mk, 128], F32, tag="pst")
                    nc.tensor.transpose(pst[:], ohc[:, t, :], identb[:])
                    ohT = ohp.tile([Mk, 128], F32, tag="ohT")
                    nc.scalar.copy(ohT[:], pst[:])
                    nc.tensor.matmul(gps[:, 4 * t:4 * (t + 1)], ohT[:],
                                     gtabt[:], start=True, stop=True)
                nc.scalar.copy(gres[:, k0 * 4:(k0 + CH) * 4], gps[:])

            produce(0)
            for c in range(1, NCH):
                produce(c)
                consume(c - 1)
            consume(NCH - 1)

            # ---- global per-GT max: partition reduce, exact scatter to the
            # full M columns (ap_gather with an inverse index map + sentinel),
            # AllReduce(max), exact gather back to kept columns ----
            cmka = colp.tile([128, Mk], F32, tag="cmka")
            nc.gpsimd.partition_all_reduce(cmka[:], cmk[:], channels=128,
                                           reduce_op=bass_isa.ReduceOp.max)
            cmext = colp.tile([128, Mk + 16], F32, tag="cmext")
            nc.vector.tensor_copy(cmext[:, 0:Mk], cmka[:])
            nc.vector.tensor_scalar(cmext[:, Mk:Mk + 16], cmka[:, 0:16],
                                    0.0, -BIG_AREA, op0=ALU.mult, op1=ALU.add)
            cfull = colp.tile([128, M], F32, tag="cfull")
            nc.gpsimd.ap_gather(cfull[:], cmext[:], invwt[:], channels=128,
                                num_elems=Mk + 16, d=1, num_idxs=M)
            nc.sync.dma_start(cm_in[:], cfull[0:1, :])
            nc.gpsimd.collective_compute(
                "AllReduce", ALU.max, replica_groups=rg,
                ins=[cm_in[:].opt()], outs=[cm_out[:].opt()])
            g1 = colp.tile([1, M], F32, tag="g1")
            nc.sync.dma_start(g1[:], cm_out[:])
            gfb = colp.tile([128, M], F32, tag="gfb")
            nc.gpsimd.partition_broadcast(gfb[:], g1[:], channels=128)
            cmaxt = colp.tile([128, Mk], F32, tag="cmaxt")
            nc.gpsimd.ap_gather(cmaxt[:], gfb[:], keptwt[:], channels=128,
                                num_elems=M, d=1, num_idxs=Mk)
            nc.vector.tensor_tensor(cmaxt[:], cmaxt[:], kbiat[:], op=ALU.add)
            cmaxb = _bk(cmaxt[:], CH)

            # ---- phase 2: is_best sweep (Pool) ----
            for c in range(NCH):
                k0 = c * CH
                gv = gbuf_t[:, k0 * Mk:(k0 + CH) * Mk].rearrange(
                    "p (k j) -> p k j", j=Mk)
                ee = ywork.tile([128, CH, Mk], F32,
                                tag="m3" if c % 2 == 0 else "m4")
                nc.vector.tensor_tensor(ee[:], gv, cmaxb, op=ALU.is_equal)
                nc.vector.reduce_max(isbb[:, k0:k0 + CH], ee[:], axis=AX.X)

            if dbg is not None:
                nc.sync.dma_start(dbg[0], vmaxb[:])
                nc.sync.dma_start(dbg[1], isbb[:])
                nc.sync.dma_start(dbg[2], vmaxb[:])
                nc.sync.dma_start(dbg[3], isbb[:])
                nc.sync.dma_start(dbg2[0], cmaxt[:])
                nc.sync.dma_start(dbg2[1], cmka[:])

            _ph.close()  # free phase pools before the tail allocations

            # ---- labels + priorities ----
            fgm = colp.tile([128, NT], F32, tag="fgm")
            tvf = colp.tile([128, NT], F32, tag="tvf")
            nc.vector.tensor_scalar(tvf[:], vmaxb[:], THR_FG, None,
                                    op0=ALU.is_ge)
            nc.vector.tensor_tensor(fgm[:], tvf[:], isbb[:], op=ALU.max)
            bgm0 = colp.tile([128, NT], F32, tag="bgm0")
            nc.vector.scalar_tensor_tensor(bgm0[:], vmaxb[:], THR_BG,
                                           insidec[:], op0=ALU.is_lt,
                                           op1=ALU.mult)
            nfgm = colp.tile([128, NT], F32, tag="nfgm")
            nc.vector.tensor_scalar(nfgm[:], fgm[:], -1.0, 1.0,
                                    op0=ALU.mult, op1=ALU.add)
            bgm = colp.tile([128, NT], F32, tag="bgm")
            nc.vector.tensor_tensor(bgm[:], bgm0[:], nfgm[:], op=ALU.mult)

            prfg = colp.tile([128, NT], F32, tag="prfg")
            s1 = colp.tile([128, NT], F32, tag="s1")
            nc.vector.scalar_tensor_tensor(s1[:], nrfgc[:], 2.0, fgm[:],
                                           op0=ALU.add, op1=ALU.mult)
            nc.vector.tensor_scalar(prfg[:], s1[:], -2.0, None, op0=ALU.add)
            prbg = colp.tile([128, NT], F32, tag="prbg")
            s2 = colp.tile([128, NT], F32, tag="s2")
            nc.vector.scalar_tensor_tensor(s2[:], nrbgc[:], 2.0, bgm[:],
                                           op0=ALU.add, op1=ALU.mult)
            nc.vector.tensor_scalar(prbg[:], s2[:], -2.0, None, op0=ALU.add)

            # ---- top-8 per lane, AllGather candidates, kth thresholds ----
            fg8 = colp.tile([128, 8], F32, tag="fg8")
            nc.vector.max(fg8[:], prfg[:])
            bg8 = colp.tile([128, 8], F32, tag="bg8")
            nc.vector.max(bg8[:], prbg[:])
            nc.sync.dma_start(ag_in[0], fg8[:])
            nc.sync.dma_start(ag_in[1], bg8[:])
            nc.gpsimd.collective_compute(
                "AllGather", ALU.bypass, replica_groups=rg,
                ins=[ag_in[:].opt()], outs=[ag_out[:].opt()])

            # gathered candidates: fg in columns 0:64, bg in 64:128
            cboth = colp.tile([128, 16 * n_cores], F32, tag="cboth")
            for r in range(n_cores):
                nc.sync.dma_start(cboth[:, r * 8:(r + 1) * 8], ag_out[r, 0])
                nc.sync.dma_start(cboth[:, 64 + r * 8:64 + (r + 1) * 8],
                                  ag_out[r, 1])

            # exact 128th-largest threshold by two rounds of ladder counting
            # (counts are global: every core holds all 1024 candidates)
            LO = LAD_LO
            ST = LAD_ST
            ST2 = ST / 64.0

            def extract_tau(cnta, base_is_lad1, tau_lo, tag):
                # cnta [128, 64] all-reduced counts -> tau [128, 1]
                mm = colp.tile([128, 64], F32, tag=f"mm{tag}")
                nc.vector.tensor_scalar(mm[:], cnta, float(NUM_FG), None,
                                        op0=ALU.is_ge)
                nc.vector.tensor_tensor(mm[:], mm[:], iot64t[:], op=ALU.mult)
                ii = colp.tile([128, 1], F32, tag=f"ii{tag}")
                nc.vector.reduce_max(ii[:], mm[:], axis=AX.X)
                tau = colp.tile([128, 1], F32, tag=f"tau{tag}")
                if base_is_lad1:
                    nc.vector.tensor_scalar(tau[:], ii[:], ST, LO,
                                            op0=ALU.mult, op1=ALU.add)
                else:
                    nc.vector.tensor_scalar(tau[:], ii[:], ST2,
                                            tau_lo[:, 0:1],
                                            op0=ALU.mult, op1=ALU.add)
                return tau

            with tc.tile_pool(name="ladp", bufs=1) as ladp:
                c1 = colp.tile([128, 128], F32, tag="c1")
                cmp1 = ladp.tile([128, 64, 64], F32, tag="cmp")
                nc.vector.tensor_tensor(cmp1[:], _bk(cboth[:, 0:64], 64),
                                        _bj(lad1t[:], 64), op=ALU.is_ge)
                nc.vector.reduce_sum(c1[:, 0:64], cmp1[:], axis=AX.X)
                cmp1b = ladp.tile([128, 64, 64], F32, tag="cmp")
                nc.vector.tensor_tensor(cmp1b[:], _bk(cboth[:, 64:128], 64),
                                        _bj(lad1t[:], 64), op=ALU.is_ge)
                nc.vector.reduce_sum(c1[:, 64:128], cmp1b[:], axis=AX.X)
                c1a = colp.tile([128, 128], F32, tag="c1a")
                nc.gpsimd.partition_all_reduce(
                    c1a[:], c1[:], channels=128,
                    reduce_op=bass_isa.ReduceOp.add)
                tlo_f = extract_tau(c1a[:, 0:64], True, None, "f1")
                tlo_b = extract_tau(c1a[:, 64:128], True, None, "b1")

                lad2f = colp.tile([128, 64], F32, tag="lad2f")
                nc.vector.tensor_scalar(lad2f[:], iot64t[:], ST2,
                                        tlo_f[:, 0:1], op0=ALU.mult,
                                        op1=ALU.add)
                lad2b = colp.tile([128, 64], F32, tag="lad2b")
                nc.vector.tensor_scalar(lad2b[:], iot64t[:], ST2,
                                        tlo_b[:, 0:1], op0=ALU.mult,
                                        op1=ALU.add)
                c2 = colp.tile([128, 128], F32, tag="c2")
                cmp2 = ladp.tile([128, 64, 64], F32, tag="cmp")
                nc.vector.tensor_tensor(cmp2[:], _bk(cboth[:, 0:64], 64),
                                        _bj(lad2f[:], 64), op=ALU.is_ge)
                nc.vector.reduce_sum(c2[:, 0:64], cmp2[:], axis=AX.X)
                cmp2b = ladp.tile([128, 64, 64], F32, tag="cmp")
                nc.vector.tensor_tensor(cmp2b[:], _bk(cboth[:, 64:128], 64),
                                        _bj(lad2b[:], 64), op=ALU.is_ge)
                nc.vector.reduce_sum(c2[:, 64:128], cmp2b[:], axis=AX.X)
                c2a = colp.tile([128, 128], F32, tag="c2a")
                nc.gpsimd.partition_all_reduce(
                    c2a[:], c2[:], channels=128,
                    reduce_op=bass_isa.ReduceOp.add)
                thfgb = extract_tau(c2a[:, 0:64], False, tlo_f, "f2")
                thbgb = extract_tau(c2a[:, 64:128], False, tlo_b, "b2")

            # counts over the gathered candidate sets -> 1 / num_examples
            mcf = colp.tile([128, 64], F32, tag="mcf")
            nc.vector.tensor_scalar(mcf[:], cboth[:, 0:64], thfgb[:, 0:1],
                                    None, op0=ALU.is_ge)
            nf1 = colp.tile([128, 1], F32, tag="nf1")
            nc.vector.reduce_sum(nf1[:], mcf[:], axis=AX.X)
            nfk = colp.tile([128, 1], F32, tag="nfk")
            nc.gpsimd.partition_all_reduce(nfk[:], nf1[:], channels=128,
                                           reduce_op=bass_isa.ReduceOp.add)
            mcb = colp.tile([128, 64], F32, tag="mcb")
            nc.vector.tensor_scalar(mcb[:], cboth[:, 64:128], thbgb[:, 0:1],
                                    None, op0=ALU.is_ge)
            nb1 = colp.tile([128, 1], F32, tag="nb1")
            nc.vector.reduce_sum(nb1[:], mcb[:], axis=AX.X)
            nbk = colp.tile([128, 1], F32, tag="nbk")
            nc.gpsimd.partition_all_reduce(nbk[:], nb1[:], channels=128,
                                           reduce_op=bass_isa.ReduceOp.add)
            numex = colp.tile([128, 1], F32, tag="numex")
            nc.vector.tensor_tensor(numex[:], nfk[:], nbk[:], op=ALU.add)
            invne = colp.tile([128, 1], F32, tag="invne")
            nc.vector.reciprocal(invne[:], numex[:])

            # ---- phase 3: final labels / weights / bbox targets ----
            mfg = colp.tile([128, NT], F32, tag="mfg")
            nc.vector.tensor_scalar(mfg[:], prfg[:], thfgb[:, 0:1], None,
                                    op0=ALU.is_ge)
            mbg = colp.tile([128, NT], F32, tag="mbg")
            nc.vector.tensor_scalar(mbg[:], prbg[:], thbgb[:, 0:1], None,
                                    op0=ALU.is_ge)
            labf = colp.tile([128, NT], F32, tag="labf")
            nc.vector.scalar_tensor_tensor(labf[:], mfg[:], 2.0, mbg[:],
                                           op0=ALU.mult, op1=ALU.add)
            nc.vector.tensor_scalar(labf[:], labf[:], 1.0, None,
                                    op0=ALU.subtract)
            oww = colp.tile([128, NT], F32, tag="oww")
            nc.vector.tensor_tensor(oww[:], mfg[:], mbg[:], op=ALU.add)
            nc.vector.tensor_scalar(oww[:], oww[:], invne[:, 0:1], None,
                                    op0=ALU.mult)

            _tp = contextlib.ExitStack()
            resp = _tp.enter_context(tc.tile_pool(name="resp", bufs=1))
            res = resp.tile([128, NT * 7], F32, tag="res")
            r3 = res[:].rearrange("p (k c) -> p k c", c=7)
            g4 = gres[:].rearrange("p (k c) -> p k c", c=4)
            # zero-overlap rows have an all-zero one-hot; blend in GT0 params
            zs = colp.tile([128, NT], F32, tag="zs")
            nc.vector.tensor_scalar(zs[:], vmaxb[:], 0.0, None, op0=ALU.is_gt)
            nzs = colp.tile([128, NT], F32, tag="nzs")
            nc.vector.tensor_scalar(nzs[:], zs[:], -1.0, 1.0,
                                    op0=ALU.mult, op1=ALU.add)
            tb1 = colp.tile([128, NT], F32, tag="tb1")
            for cc in range(4):
                nc.vector.tensor_scalar(tb1[:], nzs[:], gt0b[:, cc:cc + 1],
                                        None, op0=ALU.mult)
                nc.vector.tensor_tensor(g4[:, :, cc], g4[:, :, cc], zs[:],
                                        op=ALU.mult)
                nc.vector.tensor_tensor(g4[:, :, cc], g4[:, :, cc], tb1[:],
                                        op=ALU.add)
            tmp = colp.tile([128, NT], F32, tag="tmp")
            nc.vector.tensor_tensor(tmp[:], g4[:, :, 0], ecxc[:],
                                    op=ALU.subtract)
            nc.vector.tensor_tensor(r3[:, :, 1], tmp[:], invewc[:],
                                    op=ALU.mult)
            nc.vector.tensor_tensor(tmp[:], g4[:, :, 1], ecyc[:],
                                    op=ALU.subtract)
            nc.vector.tensor_tensor(r3[:, :, 2], tmp[:], invehc[:],
                                    op=ALU.mult)
            nc.vector.tensor_tensor(r3[:, :, 3], g4[:, :, 2], logewc[:],
                                    op=ALU.subtract)
            nc.vector.tensor_tensor(r3[:, :, 4], g4[:, :, 3], logehc[:],
                                    op=ALU.subtract)
            for cc in range(4):
                nc.vector.tensor_tensor(r3[:, :, 1 + cc], r3[:, :, 1 + cc],
                                        insidec[:], op=ALU.mult)
            nc.vector.tensor_copy(r3[:, :, 0], labf[:])
            nc.vector.tensor_copy(r3[:, :, 5], mfg[:])
            nc.vector.tensor_copy(r3[:, :, 6], oww[:])

            nc.sync.dma_start(outt[:], res[:])
            _tp.close()

    nc.compile()
    return nc


def _kept_sets(all_anchors, gt, n_cores):
    T = all_anchors.shape[0]
    TPC = T // n_cores
    gx1, gy1, gx2, gy2 = gt[:, 0], gt[:, 1], gt[:, 2], gt[:, 3]
    sets = []
    for c in range(n_cores):
        sl = slice(c * TPC, (c + 1) * TPC)
        aa = all_anchors[sl]
        keep = ((gy2 + 1 > aa[:, 1].min()) & (gy1 < aa[:, 3].max() + 1)
                & (gx2 + 1 > aa[:, 0].min()) & (gx1 < aa[:, 2].max() + 1))
        keep[0] = True
        sets.append(np.nonzero(keep)[0])
    return sets


def plan_mk(rpn_cls_score, gt_boxes, anchors, feat_stride, n_cores):
    f32 = np.float32
    H, W = rpn_cls_score.shape[-2:]
    anchors = np.asarray(anchors, dtype=f32)
    fs = f32(feat_stride)
    sx = np.arange(W, dtype=f32) * fs
    sy = np.arange(H, dtype=f32) * fs
    gy, gx = np.meshgrid(sy, sx, indexing="ij")
    shifts = np.stack([gx.ravel(), gy.ravel(), gx.ravel(), gy.ravel()],
                      axis=1).astype(f32)
    all_anchors = (anchors[None, :, :] + shifts[:, None, :]).reshape(-1, 4)
    gt = np.asarray(gt_boxes, dtype=f32)
    sets = _kept_sets(all_anchors, gt, n_cores)
    mx = max(len(s) for s in sets)
    Mk = min(M, int(np.ceil(max(mx, 32) / 16.0) * 16))
    return Mk, all_anchors, sets


def prep_inputs(rpn_cls_score, gt_boxes, im_info, anchors, rand_fg, rand_bg,
                feat_stride, n_cores, Mk=None, all_anchors=None, ksets=None):
    """Host-side input marshalling."""
    f32 = np.float32
    H, W = rpn_cls_score.shape[-2:]
    T = H * W * A
    TPC = T // n_cores
    NT = TPC // 128
    CH = _pick_ch(NT)
    if Mk is None or all_anchors is None or ksets is None:
        Mk, all_anchors, ksets = plan_mk(rpn_cls_score, gt_boxes, anchors,
                                         feat_stride, n_cores)

    ax1, ay1, ax2, ay2 = (all_anchors[:, i] for i in range(4))
    im = np.asarray(im_info, dtype=f32)[0]
    inside = ((ax1 >= 0) & (ay1 >= 0) & (ax2 < im[1]) & (ay2 < im[0]))

    ew = ax2 - ax1 + f32(1.0)
    eh = ay2 - ay1 + f32(1.0)
    a_area = ew * eh
    a_area_eff = np.where(inside, a_area, f32(BIG_AREA)).astype(f32)
    ecx = ax1 + f32(0.5) * ew
    ecy = ay1 + f32(0.5) * eh
    ckm = np.tile((f32(CH * Mk)
                   - (np.arange(NT) % CH).astype(f32) * f32(Mk)), (128, 1))

    gt = np.asarray(gt_boxes, dtype=f32)
    gx1, gy1, gx2, gy2 = gt[:, 0], gt[:, 1], gt[:, 2], gt[:, 3]
    gw = gx2 - gx1 + f32(1.0)
    gh = gy2 - gy1 + f32(1.0)
    g_area = gw * gh
    gcx = gx1 + f32(0.5) * gw
    gcy = gy1 + f32(0.5) * gh
    loggw = np.log(gw).astype(f32)
    loggh = np.log(gh).astype(f32)

    rand_fg = np.asarray(rand_fg, dtype=f32)
    rand_bg = np.asarray(rand_bg, dtype=f32)

    in_maps = []
    for c in range(n_cores):
        sl = slice(c * TPC, (c + 1) * TPC)
        idx = ksets[c]
        nk = len(idx)
        assert nk <= Mk, f"core {c}: kept {nk} > Mk {Mk}"

        coefs = np.stack([
            -ax1[sl], ax2[sl] + f32(1.0), -ay1[sl], ay2[sl] + f32(1.0),
            a_area_eff[sl],
            inside[sl].astype(f32), (f32(1.0) / ew[sl]), (f32(1.0) / eh[sl]),
            ecx[sl], ecy[sl], np.log(ew[sl]), np.log(eh[sl]),
            np.zeros(TPC, f32),  # placeholder, replaced below
            (-rand_fg[sl]), (-rand_bg[sl]),
        ], axis=0).astype(f32).reshape(NPL, 128, NT)
        coefs[12] = ckm
        coefs = coefs.transpose(1, 0, 2).reshape(128, NPL * NT)

        kx1 = np.full(Mk, f32(-1e6)); kx2 = np.full(Mk, f32(-1e6 + 1))
        ky1 = np.full(Mk, f32(-1e6)); ky2 = np.full(Mk, f32(-1e6 + 1))
        kga = np.full(Mk, f32(BIG_AREA))
        kx1[:nk], kx2[:nk] = gx1[idx], gx2[idx]
        ky1[:nk], ky2[:nk] = gy1[idx], gy2[idx]
        kga[:nk] = g_area[idx]
        gcoefs = np.concatenate([
            np.tile(-kx1, (128, 1)), np.tile(kx2 + f32(1.0), (128, 1)),
            np.tile(-ky1, (128, 1)), np.tile(ky2 + f32(1.0), (128, 1)),
            np.tile(kga, (128, 1)),
        ], axis=1).astype(f32)

        gtab = np.zeros((Mk, 4), f32)
        gtab[:nk, 0] = gcx[idx]
        gtab[:nk, 1] = gcy[idx]
        gtab[:nk, 2] = loggw[idx]
        gtab[:nk, 3] = loggh[idx]

        # inverse map: full column j -> kept slot (or the -1e30 sentinel)
        inv_full = np.full(M, Mk, np.int16)
        inv_full[idx] = np.arange(nk, dtype=np.int16)
        kept_idx = np.zeros(Mk, np.int16)
        kept_idx[:nk] = idx.astype(np.int16)
        kbias = np.zeros((128, Mk), f32)
        kbias[:, nk:] = f32(-BIG_AREA)

        def wrap16(a):
            # ap_gather idx layout: position i -> idxs[i % 16, i // 16],
            # replicated across the 8 Q7 16-partition groups
            w = a.reshape(-1, 16).T.astype(np.int16)      # [16, n/16]
            return np.tile(w, (8, 1))

        in_maps.append({
            "acoef": np.ascontiguousarray(coefs),
            "gcoef": np.ascontiguousarray(gcoefs),
            "gtab": gtab,
            "invw": wrap16(inv_full),
            "keptw": wrap16(kept_idx),
            "kbias": kbias,
            "gt0": np.array([[gcx[0], gcy[0], loggw[0], loggh[0]]], f32),
            "lad1": np.tile((f32(LAD_LO)
                             + np.arange(64, dtype=f32) * f32(LAD_ST)),
                            (128, 1)),
            "iot64": np.tile(np.arange(64, dtype=f32), (128, 1)),
        })
    return in_maps


_GRAPH_CACHE = {}


def run(inputs, n_cores=8, trace=False):
    H, W = inputs["rpn_cls_score"].shape[-2:]
    Mk, all_anchors, ksets = plan_mk(inputs["rpn_cls_score"],
                                     inputs["gt_boxes"], inputs["anchors"],
                                     inputs["feat_stride"], n_cores)
    key = (H, W, n_cores, Mk)
    if key not in _GRAPH_CACHE:
        _GRAPH_CACHE[key] = build_graph(H, W, n_cores, Mk)
    nc = _GRAPH_CACHE[key]
    in_maps = prep_inputs(
        inputs["rpn_cls_score"], inputs["gt_boxes"], inputs["im_info"],
        inputs["anchors"], inputs["rand_fg"], inputs["rand_bg"],
        inputs["feat_stride"], n_cores, Mk, all_anchors, ksets)
    res = run_bass_kernel_spmd(nc, in_maps, core_ids=list(range(n_cores)),
                               trace=trace)
    T = H * W * A
    TPC = T // n_cores
    out = np.concatenate(
        [r["out"].reshape(TPC, 7) for r in res.results], axis=0)
    return out, res


def kernel(**inputs) -> np.ndarray:
    out, _ = run(inputs, n_cores=8, trace=False)
    return out
